# revision 22
# baseline (speedup 1.0000x reference)
"""HeterogeneousKANLayer forward on 8 Trainium2 NeuronCores.

Math (reference):
  xn    = tanh(x)                                  [B, I]
  base  = silu(xn)                                 [B, I]
  basis = exp(-((xn - c_j)/w)^2), c_j evenly spaced on [-1,1], w = 2/(C-1)
  out[b,o] = sum_{i,c} basis[b,i,c]*coef[i,o,c]*scale_sp[o,i]
           + sum_i base[b,i]*scale_base[o,i]

Kernel strategy (data-parallel over batch, 8 cores x 512 rows):
  One [512b, 5632k] @ [5632k, 512o] contraction per core.
  The 10 Gaussian-center channel groups run as fp8e4 DoubleRow matmuls
  (K=256 per instruction, ~2x PE rate); the silu residual group runs as
  fp16 matmuls. Planes are stored scaled by S_G to keep the fp8 band
  normal; 1/S_G is folded into the PSUM drain.
  Basis production, balanced ~equally over ACT and DVE:
    - anchors 0 and 8 fully on ACT: Square(xn - c_j), Exp(-20.25*sq).
    - anchors 3 and 6 via DVE: q_j = (a_j - a_0)*xn + sq0n with
      sq0n = -20.25*sq0 - b_0 (tensor_scalar, 4x DVE mode), then
      g_j = Exp(q_j + b_j + lnS) on ACT directly to fp8.
    - chains: g_{j+1} = (r * e^{8-2j}) * g_j on DVE (stt), with
      r = exp(9*xn); evenly spaced centers make consecutive Gaussians
      differ by exp(9*xn + const). Chains 1,2 are emitted before the
      q's so the PE's early planes aren't queued behind them.
  silu = 0.5*xn*(1+tanh(xn/2)): tanh shares the exp ACT table set, so
  the whole kernel needs ONE table load; the 0.5 folds into ws.
  A dummy-matmul warm burst keeps the PE HAM clock at 2.4 GHz through
  the production-paced head. PE consumes all pair-0 halves first, then
  pair-1, so the batch-lagged second half never stalls the stream.
"""

import sys
import types

import numpy as np
import ml_dtypes

import concourse.bass as bass
import concourse.tile as tile
from concourse import bacc, mybir

N_CORES = 8
B = 4096
I = 512
O = 512
C = 10
BS = B // N_CORES          # batch rows per core (512)
NT = I // 128              # 4 i-tiles
W_SP = 2.0 / (C - 1)       # rbf width == center spacing (2/9)
IW2 = 1.0 / (W_SP * W_SP)  # 20.25
CENTERS = np.linspace(-1.0, 1.0, C)
A_J = 2.0 * CENTERS * IW2
B_J = -(CENTERS ** 2) * IW2
DB = B_J[1:] - B_J[:-1]              # 8,6,4,2,0,-2,-4,-6,-8
S_G = 64.0                           # fp8 plane scale
LN_S = float(np.log(S_G))

SQ_ANCHORS = [0, 3]                  # fully on ACT
Q_ANCHORS = [6, 8]                   # DVE q + ACT Exp
CHAINS_EARLY = [(1, 0), (2, 1)]      # feed the PE head; emitted first
CHAINS_LATE = [(4, 3), (5, 4), (7, 6), (9, 8)]
N_WARM_MM = 6

_CACHE = {}
_DEBUG_TAPS = False


def _build(rank1):
    """Build and finalize the per-core Bass module (same on all cores)."""
    nc = bacc.Bacc("TRN2", target_bir_lowering=False, debug=False,
                   num_devices=N_CORES)
    f32 = mybir.dt.float32
    f16 = mybir.dt.float16
    fp8 = mybir.dt.float8e4
    DR = mybir.MatmulPerfMode.DoubleRow
    MUL = mybir.AluOpType.mult
    ADD = mybir.AluOpType.add
    EXP = mybir.ActivationFunctionType.Exp
    TANH = mybir.ActivationFunctionType.Tanh
    SQUARE = mybir.ActivationFunctionType.Square

    xt_d = nc.dram_tensor("xt", (128, NT, BS), f16, kind="ExternalInput")
    wf_d = nc.dram_tensor("wf", (128, 4 * C, O), fp8, kind="ExternalInput")
    ws_d = nc.dram_tensor("ws", (128, NT, O), f16, kind="ExternalInput")
    out_d = nc.dram_tensor("out", (BS, O), f32, kind="ExternalOutput")

    with tile.TileContext(nc) as tc:
        with (
            tc.tile_pool(name="big", bufs=1) as big,
            tc.tile_pool(name="wpool", bufs=1) as wpool,
            tc.tile_pool(name="psum", bufs=1, space="PSUM") as psum,
        ):
            xt_sb = big.tile([128, NT, BS], f16, tag="xt")
            xn = big.tile([128, NT, BS], f16, tag="xn")
            sqa = {j: big.tile([128, NT, BS], f16, name=f"sq{j}")
                   for j in SQ_ANCHORS}
            sq0n = big.tile([128, NT, BS], f16, tag="sq0n")
            qs = {j: big.tile([128, NT, BS], f16, name=f"q{j}")
                  for j in Q_ANCHORS}
            r_t = big.tile([128, NT, BS], f16, tag="r")
            th2 = big.tile([128, NT, BS], f16, tag="th2")
            silu = big.tile([128, NT, BS], f16, tag="silu")
            g = [big.tile([128, NT, BS], fp8, name=f"g{j}") for j in range(C)]
            wf_sb = wpool.tile([128, 4 * C, O], fp8, name="wf_sb")
            ws_sb = wpool.tile([128, NT, O], f16, name="ws_sb")
            warm = big.tile([128, 8], f32, tag="warm")
            dwarm = big.tile([128, 1], f16, tag="dwarm")
            wmm_s = big.tile([128, 2, 128], fp8, tag="wmm_s")
            wmm_m = big.tile([128, 2, 512], fp8, tag="wmm_m")
            # bias columns: [0]=-c_0, [1]=-c_8 (Square shifts),
            # [2]=lnS (sq-anchor Exp), [3..]= b_j + lnS for q-anchors
            bias_sb = big.tile([128, 3 + len(Q_ANCHORS)], f32, tag="bias")
            nc.vector.memset(bias_sb[:, 0:1], float(-CENTERS[0]))
            nc.vector.memset(bias_sb[:, 1:2], float(-CENTERS[3]))
            nc.vector.memset(bias_sb[:, 2:3], LN_S)
            for ai, j in enumerate(Q_ANCHORS):
                nc.vector.memset(bias_sb[:, 3 + ai:4 + ai],
                                 float(B_J[j] + LN_S))

            def hs(h):
                return slice(2 * h, 2 * h + 2)

            # warm-ups: ACT table load at t=0 + dummy tiles for PE burst
            nc.vector.memset(warm[:], 0.0)
            nc.scalar.activation(out=warm[:], in_=warm[:], func=EXP)
            nc.scalar.activation(out=warm[:], in_=warm[:], func=TANH)
            nc.vector.memset(wmm_s[:], 0.0)
            nc.vector.memset(wmm_m[:], 0.0)

            # ---- DMAs (single SP queue; bandwidth-ordered) ----
            nc.sync.dma_start(out=dwarm[:, 0:1], in_=xt_d[0:128, 0:1, 0:1])
            for t in (0, 1):
                nc.sync.dma_start(out=xt_sb[:, t:t + 1, :],
                                  in_=xt_d[:, t:t + 1, :])
            # center-0 weights first so the first matmul isn't DMA-gated
            nc.sync.dma_start(out=wf_sb[:, 0:4, :], in_=wf_d[:, 0:4, :])
            for t in (2, 3):
                nc.sync.dma_start(out=xt_sb[:, t:t + 1, :],
                                  in_=xt_d[:, t:t + 1, :])
            nc.sync.dma_start(out=wf_sb[:, 4:12, :], in_=wf_d[:, 4:12, :])
            nc.sync.dma_start(out=ws_sb[:, :, :], in_=ws_d[:, :, :])
            for (k0, k1) in [(12, 20), (20, 28), (28, 40)]:
                nc.sync.dma_start(out=wf_sb[:, k0:k1, :],
                                  in_=wf_d[:, k0:k1, :])

            # ---- PE warm burst: hold HAM at 2.4GHz through the head ----
            ps = [psum.tile([128, O], f32, name=f"ps{bt}") for bt in range(4)]
            warm_ps = psum.tile([128, O], f32, name="warm_ps")
            for _ in range(N_WARM_MM):
                nc.tensor.matmul(warm_ps, wmm_s[:, :, :], wmm_m[:, :, :],
                                 start=True, stop=True, perf_mode=DR)

            # ---- production (program order == scheduler priority) ----
            def act(out, in_, func, h, **kw):
                nc.scalar.activation(out=out[:, hs(h), :],
                                     in_=in_[:, hs(h), :], func=func, **kw)

            def stt(out, in0, scalar, in1, h, op0=MUL, op1=MUL):
                nc.vector.scalar_tensor_tensor(
                    out=out[:, hs(h), :], in0=in0[:, hs(h), :],
                    scalar=scalar, in1=in1[:, hs(h), :], op0=op0, op1=op1)

            def chain(j, src, h):
                stt(g[j], r_t, float(np.exp(DB[src])), g[src], h)

            def sq_anchor(j, bias_col, h):
                act(sqa[j], xn, SQUARE, h, bias=bias_sb[:, bias_col:bias_col + 1])
                act(g[j], sqa[j], EXP, h, scale=float(-IW2),
                    bias=bias_sb[:, 2:3])

            # --- half A: ACT head then DVE block, q-exps for A first ---
            for t in (0, 1):
                nc.scalar.activation(out=xn[:, t:t + 1, :],
                                     in_=xt_sb[:, t:t + 1, :], func=TANH)
            sq_anchor(0, 0, 0)
            act(r_t, xn, EXP, 0, scale=float(2.0 / W_SP))
            sq_anchor(3, 1, 0)
            chain(1, 0, 0)
            chain(2, 1, 0)
            nc.vector.tensor_scalar(
                out=sq0n[:, hs(0), :], in0=sqa[0][:, hs(0), :],
                scalar1=float(-IW2), scalar2=float(-B_J[0]),
                op0=MUL, op1=ADD)
            for j in Q_ANCHORS:
                stt(qs[j], xn, float(A_J[j] - A_J[0]), sq0n, 0,
                    op0=MUL, op1=ADD)
            chain(4, 3, 0)
            chain(5, 4, 0)
            for j in Q_ANCHORS:
                ai = 3 + Q_ANCHORS.index(j)
                act(g[j], qs[j], EXP, 0, bias=bias_sb[:, ai:ai + 1])
            chain(7, 6, 0)
            chain(9, 8, 0)
            # --- half B ---
            for t in (2, 3):
                nc.scalar.activation(out=xn[:, t:t + 1, :],
                                     in_=xt_sb[:, t:t + 1, :], func=TANH)
            sq_anchor(0, 0, 1)
            act(r_t, xn, EXP, 1, scale=float(2.0 / W_SP))
            sq_anchor(3, 1, 1)
            chain(1, 0, 1)
            chain(2, 1, 1)
            nc.vector.tensor_scalar(
                out=sq0n[:, hs(1), :], in0=sqa[0][:, hs(1), :],
                scalar1=float(-IW2), scalar2=float(-B_J[0]),
                op0=MUL, op1=ADD)
            for j in Q_ANCHORS:
                stt(qs[j], xn, float(A_J[j] - A_J[0]), sq0n, 1,
                    op0=MUL, op1=ADD)
            chain(4, 3, 1)
            chain(5, 4, 1)
            for j in Q_ANCHORS:
                ai = 3 + Q_ANCHORS.index(j)
                act(g[j], qs[j], EXP, 1, bias=bias_sb[:, ai:ai + 1])
            chain(7, 6, 1)
            chain(9, 8, 1)
            # --- silu last on ACT (second table set loads here) ---
            for h in (0, 1):
                act(silu, xn, mybir.ActivationFunctionType.Silu, h)

            # ---- PE: pair-0 pass, pair-1 pass, then silu as the stop ----
            def mm_g(j, p, start=False, stop=False):
                for bt in range(4):
                    nc.tensor.matmul(
                        ps[bt],
                        g[j][:, 2 * p:2 * p + 2, bt * 128:(bt + 1) * 128],
                        wf_sb[:, 4 * j + 2 * p:4 * j + 2 * p + 2, :],
                        start=start, stop=stop, perf_mode=DR)

            ORDER = [0, 1, 2, 3, 4, 5, 6, 8, 7, 9]
            mm_g(0, 0, start=True)
            for j in ORDER[1:]:
                mm_g(j, 0)
            for j in ORDER:
                mm_g(j, 1)
            for t in (0, 1, 2):
                for bt in range(4):
                    nc.tensor.matmul(
                        ps[bt], silu[:, t, bt * 128:(bt + 1) * 128],
                        ws_sb[:, t, :], start=False, stop=False)
            # silu t3 bank-major with stop so bank bt drains early;
            # out-DMAs split across the SP and ACT hardware queues
            inv_s = float(1.0 / S_G)
            for bt in range(4):
                nc.tensor.matmul(
                    ps[bt], silu[:, 3, bt * 128:(bt + 1) * 128],
                    ws_sb[:, 3, :], start=False, stop=True)
                o_sb = big.tile([128, O], f32, name=f"o{bt}")
                if bt % 2 == 0:
                    nc.scalar.mul(out=o_sb[:], in_=ps[bt][:], mul=inv_s)
                    nc.sync.dma_start(out=out_d[bt * 128:(bt + 1) * 128, :],
                                      in_=o_sb[:])
                else:
                    nc.vector.tensor_scalar_mul(out=o_sb[:], in0=ps[bt][:],
                                                scalar1=inv_s)
                    nc.scalar.dma_start(out=out_d[bt * 128:(bt + 1) * 128, :],
                                        in_=o_sb[:])
            if _DEBUG_TAPS:
                taps = {"g0": g[0], "g1": g[1], "g5": g[5], "g9": g[9],
                        "silu": silu, "r": r_t}
                for nm, t_sb in taps.items():
                    d_out = nc.dram_tensor(f"dbg_{nm}", (128, NT, BS),
                                           t_sb.dtype, kind="ExternalOutput")
                    nc.sync.dma_start(out=d_out[:, :, :], in_=t_sb[:, :, :])
    nc.finalize()
    return nc


def _prep_inputs(x, coef, scale_base, scale_sp):
    """Host-side shard + layout prep (cheap numpy reshapes/casts)."""
    x = np.asarray(x, dtype=np.float32)
    coef = np.asarray(coef, dtype=np.float32)
    scale_base = np.asarray(scale_base, dtype=np.float32)
    scale_sp = np.asarray(scale_sp, dtype=np.float32)

    # wf[p, kt, o] (partition-major for contiguous DMA), kt = 4*j + t.
    wfull = coef * scale_sp.T[:, :, None]                    # [I, O, C]
    wfull = wfull.reshape(NT, 128, O, C).transpose(3, 0, 1, 2)  # [C,NT,128,O]
    wf = np.clip(wfull.reshape(4 * C, 128, O), -240.0, 240.0).astype(
        ml_dtypes.float8_e4m3).transpose(1, 0, 2)            # [128, 4C, O]
    wf = np.ascontiguousarray(wf)
    # silu' = 2*silu and the PSUM carries S_G: fold 0.5*S_G into ws.
    ws = np.ascontiguousarray(
        (S_G * scale_base.T.reshape(NT, 128, O)).transpose(1, 0, 2)
    ).astype(np.float16)

    in_maps = []
    for k in range(N_CORES):
        xs = x[k * BS:(k + 1) * BS, :]                       # [BS, I]
        xt = np.ascontiguousarray(
            xs.T.reshape(NT, 128, BS).transpose(1, 0, 2)).astype(np.float16)
        in_maps.append({"xt": xt, "wf": wf, "ws": ws})
    return in_maps, True


def _run(in_maps, rank1, trace=False):
    if "antenv.axon_hooks" not in sys.modules:
        try:
            from trn_agent_boot.trn_boot import _ntff_profile_via_ctypes
            _hook = _ntff_profile_via_ctypes("/opt/axon/libaxon_pjrt.so")
            _mod = types.ModuleType("antenv.axon_hooks")
            _mod.get_axon_ntff_profile_hook = lambda: _hook
            sys.modules["antenv.axon_hooks"] = _mod
        except Exception:
            pass
    from concourse.bass_utils import run_bass_kernel_spmd

    key = ("nc", _DEBUG_TAPS)
    if key not in _CACHE:
        _CACHE[key] = _build(True)
    return run_bass_kernel_spmd(_CACHE[key], in_maps,
                                core_ids=list(range(N_CORES)), trace=trace)


def kernel(x, coef, scale_base, scale_sp):
    in_maps, rank1 = _prep_inputs(x, coef, scale_base, scale_sp)
    res = _run(in_maps, rank1, trace=False)
    out = np.concatenate([res.results[k]["out"] for k in range(N_CORES)],
                         axis=0)
    return out.astype(np.float32)
